# revision 54
# baseline (speedup 1.0000x reference)
"""Cached multi-head attention, head-sharded (tensor-parallel) over 8 NeuronCores.

Per core: 2 of 16 heads. Q/K/V projections with column-sharded weights,
flash-style attention in S^T layout (keys on partitions), partial Wo
projection with row-sharded Wo; partials are summed on the host.

Single interleaved instruction stream: attention rounds (exp-paced on the
scalar engine) are back-filled with projection / output-projection matmuls
so the tensor engine never idles. Attention output is pre-normalized
(po * broadcast(1/rowsum)) so the Wo stage is a plain matmul + cast + DMA.
"""
import sys
import types

sys.path.insert(0, "/opt/trn_rl_repo")

# Provide antenv.axon_hooks (missing in this image) so trace=True works.
try:
    import antenv.axon_hooks  # noqa: F401
except ImportError:
    try:
        import antenv
        from trn_agent_boot.trn_boot import _ntff_profile_via_ctypes

        _mod = types.ModuleType("antenv.axon_hooks")
        _hook = _ntff_profile_via_ctypes("/opt/axon/libaxon_pjrt.so")
        _mod.get_axon_ntff_profile_hook = lambda: _hook
        _mod.set_axon_ntff_profile_hook = lambda h: None
        sys.modules["antenv.axon_hooks"] = _mod
        antenv.axon_hooks = _mod
    except Exception:
        pass

import numpy as np
import concourse.bass as bass  # noqa: F401
import concourse.bass_isa as bass_isa  # noqa: F401
from concourse import bacc
import concourse.mybir as mybir
import concourse.tile as tile
from concourse.bass_utils import run_bass_kernel_spmd

F32 = mybir.dt.float32
F16 = mybir.dt.float16
EXP = mybir.ActivationFunctionType.Exp

P = 128
B = 2
NCORES = 8
HPC = 2              # heads per core
D = 2048             # d_model
DK = 128             # head dim
EC = HPC * DK        # 256 output dims per core
SN = 2048            # new tokens
SP = 2048            # past tokens
STOT = SN + SP       # 4096 total keys
DT = D // P          # 16 d-tiles
SCH = 512            # phase-1 s-chunk
NSC = SN // SCH      # 4 chunks per batch
QCH = 512            # q chunk
NKT = STOT // P      # 32 k tiles
PKT = SP // P        # 16 past k tiles
SCALE = float(1.0 / np.sqrt(DK))

_CACHED_NC = None


def _build():
    nc = bacc.Bacc("TRN2", target_bir_lowering=False, debug=False, num_devices=NCORES)

    xT = nc.dram_tensor("xT", [B, D, SN], F16, kind="ExternalInput")
    wqT = nc.dram_tensor("wqT", [D, EC], F16, kind="ExternalInput")
    wkT = nc.dram_tensor("wkT", [D, EC], F16, kind="ExternalInput")
    wvT = nc.dram_tensor("wvT", [D, EC], F16, kind="ExternalInput")
    woT = nc.dram_tensor("woT", [EC, D], F16, kind="ExternalInput")
    # packed per-k-tile records: [.., kt, 128, 0:128]=K^T tile, [.., 128:256]=V tile
    pkv = nc.dram_tensor("pkv", [B, HPC, PKT, P, 2 * DK], F16, kind="ExternalInput")
    tri_d = nc.dram_tensor("tri", [P, P], F16, kind="ExternalInput")
    ones_d = nc.dram_tensor("ones", [P, P], F16, kind="ExternalInput")
    out = nc.dram_tensor("out", [B, SN, D], F16, kind="ExternalOutput")

    from contextlib import ExitStack
    with tile.TileContext(nc) as tc, ExitStack() as stack:
        cpool = stack.enter_context(tc.tile_pool(name="const", bufs=1))
        qt_pool = stack.enter_context(tc.tile_pool(name="qt", bufs=2))
        kvsb_pool = stack.enter_context(tc.tile_pool(name="kvsb", bufs=2))
        ot_pool = stack.enter_context(tc.tile_pool(name="ot", bufs=2))
        xt_pool = stack.enter_context(tc.tile_pool(name="xt", bufs=2))
        pt_pool = stack.enter_context(tc.tile_pool(name="pt", bufs=4))
        acc_pool = stack.enter_context(tc.tile_pool(name="accp", bufs=2))
        rT_pool = stack.enter_context(tc.tile_pool(name="rTp", bufs=2))
        o_pool = stack.enter_context(tc.tile_pool(name="outp", bufs=3))
        # PSUM: fill 2x2KB + scores 2x4KB + po 1x4KB = 16KB/partition (exact)
        fillp = stack.enter_context(tc.tile_pool(name="fill", bufs=2, space="PSUM"))
        scp = stack.enter_context(tc.tile_pool(name="scp", bufs=2, space="PSUM"))
        pop = stack.enter_context(tc.tile_pool(name="pop", bufs=1, space="PSUM"))

        t_wq = cpool.tile([P, DT, EC], F16, tag="wq")
        t_wk = cpool.tile([P, DT, EC], F16, tag="wk")
        t_wv = cpool.tile([P, DT, EC], F16, tag="wv")
        t_woT = cpool.tile([P, HPC, D], F16, tag="woT")
        t_tri = cpool.tile([P, P], F16, tag="tri")
        t_onesb = cpool.tile([P, P], F16, tag="ones")

        qt = {}
        kv_sb = {}
        ot = {}
        xtiles = {}

        def load_w(t_w, w_d, eng=None, widths=(4, 4, 4, 4)):
            eng = eng or nc.sync
            wr = w_d.rearrange("(t p) e -> p t e", p=P)
            d0 = 0
            for w in widths:
                eng.dma_start(t_w[:, d0:d0 + w, :], wr[:, d0:d0 + w, :])
                d0 += w

        def dma_x(b, c, widths=(2,) * 8):
            t = xt_pool.tile([P, DT, SCH], F16, tag="xt", name=f"xt{b}_{c}")
            xr = xT[b].rearrange("(t p) s -> p t s", p=P)
            s0 = c * SCH
            d0 = 0
            for w in widths:
                nc.sync.dma_start(
                    t[:, d0:d0 + w, :], xr[:, d0:d0 + w, s0:s0 + SCH])
                d0 += w
            xtiles[(b, c)] = t

        # ---------------- filler queue ----------------
        fillq = []
        sched = {"attn": False}  # True while dripping inside an exp-paced round

        def drip(n=1):
            done = 0
            while fillq and done < n:
                try:
                    next(fillq[0])
                    done += 1
                except StopIteration:
                    fillq.pop(0)

        def flush(g):
            while g in fillq:
                drip(64)

        # ---------------- phase 1 ----------------
        def gen_ph1(b, chunks, tail=None):
            """Q/K/V projections for `chunks` of batch b; yields between
            small PE units so attention rounds can interleave."""
            if chunks[0] == 0:
                qt[b] = qt_pool.tile([P, HPC, SN], F16, tag="qt", name=f"qt{b}")
                kv_sb[b] = kvsb_pool.tile(
                    [P, HPC, NKT, 2 * DK], F16, tag="kvsb", name=f"kvsb{b}")
                ot[b] = ot_pool.tile([P, HPC, SN], F16, tag="ot", name=f"ot{b}")

            def emit_pkv(h, eng=None):
                eng = eng or nc.sync
                for kt in range(PKT):
                    eng.dma_start(kv_sb[b][:, h, kt, :], pkv[b, h, kt])
                    if kt % 4 == 3:
                        yield

            def q_group(c, h):
                xt_ = xtiles[(b, c)]
                s0 = c * SCH
                e0 = h * DK
                psq = fillp.tile([P, SCH], F32, tag="fill", name="psq")
                for dt in range(DT):
                    nc.tensor.matmul(
                        psq, t_wq[:, dt, e0:e0 + DK], xt_[:, dt, :],
                        start=(dt == 0), stop=(dt == DT - 1))
                    if dt % 2 == 1:
                        yield
                nc.scalar.copy(qt[b][:, h, s0:s0 + SCH], psq)
                yield

            def k_group(c, h):
                xt_ = xtiles[(b, c)]
                kt0 = PKT + c * (SCH // P)
                e0 = h * DK
                psk = fillp.tile([P, SCH // P, P], F32, tag="fill", name="psk")
                for dt in range(DT):
                    nc.tensor.matmul(
                        psk[:, :, :], t_wk[:, dt, e0:e0 + DK], xt_[:, dt, :],
                        start=(dt == 0), stop=(dt == DT - 1))
                    if dt % 2 == 1:
                        yield
                nc.scalar.copy(
                    kv_sb[b][:, h, kt0:kt0 + SCH // P, 0:DK], psk)
                yield

            def v_group(c):
                xt_ = xtiles[(b, c)]
                kt0 = PKT + c * (SCH // P)
                for sub in range(SCH // P):
                    psv = fillp.tile([P, HPC, DK], F32, tag="fill", name="psv")
                    for dt in range(DT):
                        nc.tensor.matmul(
                            psv[:, :, :], xt_[:, dt, sub * P:(sub + 1) * P],
                            t_wv[:, dt, :],
                            start=(dt == 0), stop=(dt == DT - 1))
                        if dt % 2 == 1:
                            yield
                    nc.vector.tensor_copy(
                        kv_sb[b][:, 0:HPC, kt0 + sub, DK:2 * DK], psv)
                    yield

            if b == 0 and chunks[0] == 0:
                # startup: all Q groups first (only wq is resident), then K
                # groups (wk lands meanwhile), then V (wv lands meanwhile).
                # wk/wv/pkv ride the Activation DGE queue, issued up front
                # (no data deps) parallel to the x issues on the SP queue.
                def wkv_loads():
                    load_w(t_wk, wkT, eng=nc.scalar)
                    load_w(t_wv, wvT, eng=nc.scalar)
                    yield

                def prefetch2():
                    dma_x(0, 2)
                    yield
                seq = [wkv_loads(), q_group(0, 0), q_group(0, 1),
                       emit_pkv(0, nc.scalar),
                       q_group(1, 0), q_group(1, 1), k_group(0, 0),
                       k_group(0, 1), prefetch2(), k_group(1, 0),
                       k_group(1, 1), v_group(0), v_group(1)]
            else:
                seq = []
                for i, c in enumerate(chunks):
                    def prefetch(i=i):
                        nxt = (b, chunks[i + 1]) if i + 1 < len(chunks) else None
                        if nxt is not None and nxt not in xtiles:
                            dma_x(*nxt)
                        yield

                    seq.append(prefetch())
                    # h1's past KV must be emitted before round (b,h1,pr0),
                    # which can begin while chunk 2 is still dripping
                    if c == 2:
                        seq.append(emit_pkv(1, nc.scalar if b == 0 else None))
                    seq += [q_group(c, 0), k_group(c, 0), q_group(c, 1),
                            k_group(c, 1), v_group(c)]
                    if c == 0:
                        seq.append(emit_pkv(0))
            for g in seq:
                yield from g
            if tail is not None:
                for fn in tail:
                    fn()
                    yield

        # ---------------- attention ----------------
        def emit_round(b, h, pr, at_iter=None, drip_n=2):
            """One attention round: 1024 queries (chunks 2pr,2pr+1), flash
            over past+causal keys, S^T layout."""
            qA, qB = 2 * pr, 2 * pr + 1
            q0 = pr * 2 * QCH
            dA = PKT + 4 * qA + 4
            dB = PKT + 4 * qB + 4
            acc = acc_pool.tile([P, 2 * QCH], F16, tag="acc", name="acc")
            po = pop.tile([P, 2 * QCH], F32, tag="po", name="po")
            pend = []
            state = {"fa": True, "fb": True}

            def finish_half(half):
                """Denominators for one 512-query half: ones[P,128] matmul
                sums over key-partitions AND broadcasts across partitions in
                one PE op; fast reciprocal; normalize po into ot."""
                o0 = half * QCH
                sm = fillp.tile([P, QCH], F32, tag="fill", name="sm")
                nc.tensor.matmul(
                    sm, t_onesb, acc[:, o0:o0 + QCH], start=True, stop=True)
                rT = rT_pool.tile([P, QCH], F32, tag="rT", name="rT")
                nc.vector.reciprocal_approx_fast(rT, sm)
                nc.vector.tensor_mul(
                    ot[b][:, h, q0 + o0:q0 + o0 + QCH], po[:, o0:o0 + QCH], rT)

            def do_pv(item, last):
                kt, pt, v_t, aval, soA, soB, lo, _ = item
                lastA = last or kt == dA - 1
                if aval:
                    nc.tensor.matmul(
                        po[:, soA:QCH], v_t, pt[:, soA:QCH],
                        start=state["fa"], stop=lastA, skip_group_check=True)
                    state["fa"] = False
                    if lastA:
                        finish_half(0)
                nc.tensor.matmul(
                    po[:, QCH + soB:], v_t, pt[:, QCH + soB:],
                    start=state["fb"], stop=last, skip_group_check=True)
                state["fb"] = False
                if last:
                    finish_half(1)

            sched["attn"] = True
            for kt in range(dB):
                if at_iter and kt in at_iter:
                    at_iter[kt]()
                kt_t = kv_sb[b][:, h, kt, 0:P]
                v_t = kv_sb[b][:, h, kt, DK:2 * DK]
                oA = kt - (PKT + 4 * qA)
                oB = kt - (PKT + 4 * qB)
                aval = kt < dA
                soA = oA * P if 0 < oA < 4 else 0
                soB = oB * P if 0 < oB < 4 else 0
                lo = soA if aval else QCH + soB
                ps = scp.tile([P, 2 * QCH], F32, tag="ps", name="ps")
                pt = pt_pool.tile([P, 2 * QCH], F16, tag="pt", name="pt")
                if aval:
                    nc.tensor.matmul(
                        ps[:, soA:QCH], kt_t, qt[b][:, h, q0 + soA:q0 + QCH],
                        start=True, stop=True)
                nc.tensor.matmul(
                    ps[:, QCH + soB:], kt_t,
                    qt[b][:, h, q0 + QCH + soB:q0 + 2 * QCH],
                    start=True, stop=True)
                nc.scalar.activation(pt[:, lo:], ps[:, lo:], EXP, scale=SCALE)
                if 0 <= oA < 4:
                    nc.vector.tensor_mul(
                        pt[:, soA:soA + P], pt[:, soA:soA + P], t_tri)
                if 0 <= oB < 4:
                    nc.vector.tensor_mul(
                        pt[:, QCH + soB:QCH + soB + P],
                        pt[:, QCH + soB:QCH + soB + P], t_tri)
                if kt == 0:
                    nc.vector.tensor_copy(acc, pt)
                else:
                    nc.vector.tensor_add(acc[:, lo:], acc[:, lo:], pt[:, lo:])
                pend.append((kt, pt, v_t, aval, soA, soB, lo, kt == 0))
                if len(pend) > 1:
                    do_pv(pend.pop(0), False)
                drip(8 if kt == 0 else drip_n)
            do_pv(pend.pop(0), True)
            sched["attn"] = False

        # ---------------- Wo projection ----------------
        def gen_wo(b, sts, engs, rings=None, dma_engs=None):
            k = 0
            for st in sts:
                r0 = st * P
                outt = o_pool.tile([P, D], F16, tag="outt", name="outt")
                for ec in range(D // QCH):
                    e0 = ec * QCH
                    if rings is None:
                        pso = fillp.tile([P, QCH], F32, tag="fill", name="pso")
                    else:
                        # tail only: borrow the idle score ring for pipeline depth
                        pool, tg = rings[k % len(rings)]
                        pso = pool.tile([P, QCH], F32, tag=tg, name="pso")
                    nc.tensor.matmul(
                        pso, ot[b][:, 0, r0:r0 + P],
                        t_woT[:, 0, e0:e0 + QCH], start=True, stop=False)
                    nc.tensor.matmul(
                        pso, ot[b][:, 1, r0:r0 + P],
                        t_woT[:, 1, e0:e0 + QCH], start=False, stop=True)
                    yield
                    if engs is None:
                        # scalar engine is exp-saturated inside attention
                        # rounds; it is free in PE-bound seams
                        eng = "dve" if sched["attn"] else ("act", "dve")[k % 2]
                    else:
                        eng = engs[k % len(engs)]
                    k += 1
                    if eng == "act":
                        nc.scalar.copy(outt[:, e0:e0 + QCH], pso)
                    else:
                        nc.vector.tensor_copy(outt[:, e0:e0 + QCH], pso)
                    yield
                # one 512KB DMA per token-row block instead of 4x128KB
                if dma_engs is not None:
                    deng = dma_engs[st % len(dma_engs)]
                else:
                    deng = nc.sync if sched["attn"] else (nc.sync, nc.scalar)[st % 2]
                deng.dma_start(out[b, r0:r0 + P, :], outt)

        def load_woT():
            wor = woT.rearrange("(h p) d -> p h d", p=P)
            for hh in range(HPC):
                for piece in range(2):
                    e0p = piece * (D // 2)
                    nc.sync.dma_start(
                        t_woT[:, hh, e0p:e0p + D // 2],
                        wor[:, hh, e0p:e0p + D // 2])

        def load_consts():
            nc.scalar.dma_start(t_tri, tri_d[:, :])
            nc.scalar.dma_start(t_onesb, ones_d[:, :])

        # ---------------- schedule ----------------
        # PE p-state warmup: dependency-free matmuls on an uninitialized
        # const tile ramp the clock while the first DMAs land
        t_warm = cpool.tile([P, P], F16, tag="warm")
        nc.scalar.memzero(t_warm)
        ps_warm = fillp.tile([P, QCH], F32, tag="fill", name="ps_warm")
        for _ in range(24):
            nc.tensor.matmul(ps_warm[:, 0:P], t_warm, t_warm, start=True, stop=True)

        load_w(t_wq, wqT, widths=(2, 2, 4, 4, 4))
        dma_x(0, 0, widths=(1, 1, 2, 2, 2, 2, 2, 2, 2))
        dma_x(0, 1)
        load_consts()

        gA = gen_ph1(0, [0, 1])
        fillq.append(gA)
        flush(gA)

        g23 = gen_ph1(0, [2, 3], tail=[lambda: dma_x(1, 0)])
        fillq.append(g23)
        emit_round(0, 0, 0)
        emit_round(0, 1, 0)
        flush(g23)

        g1a = gen_ph1(1, [0, 1], tail=[lambda: dma_x(1, 2), load_woT])
        fillq.append(g1a)
        emit_round(0, 0, 1)
        g1b = gen_ph1(1, [2, 3])
        fillq.append(g1b)
        emit_round(0, 1, 1)
        flush(g1a)

        w0a = gen_wo(0, range(0, 8), engs=None)
        fillq.append(w0a)
        emit_round(1, 0, 0)
        flush(g1b)
        w0b = gen_wo(0, range(8, 16), engs=None)
        fillq.append(w0b)
        emit_round(1, 1, 0)

        w1a = gen_wo(1, range(0, 8), engs=None)
        fillq.append(w1a)
        emit_round(1, 0, 1)
        # st 8-11 only needs the A-half (q 1024-1535) of both pr1 rounds,
        # which completes at iter 28 of the last round — inject early so the
        # tail shrinks to the final 16 tiles
        w1b_a = gen_wo(1, range(8, 12), engs=["dve", "act"],
                       rings=[(fillp, "fill"), (scp, "ps")],
                       dma_engs=[nc.sync, nc.scalar])
        w1b_b = gen_wo(1, range(12, 16), engs=["act", "dve"],
                       rings=[(fillp, "fill"), (scp, "ps"), (pop, "po")],
                       dma_engs=[nc.sync, nc.scalar])
        emit_round(1, 1, 1, at_iter={29: lambda: fillq.append(w1b_a)})
        fillq.append(w1b_b)
        while fillq:
            drip(64)

    nc.compile()
    return nc


def _get_nc():
    global _CACHED_NC
    if _CACHED_NC is None:
        _CACHED_NC = _build()
    return _CACHED_NC


def pack_kv(pk, pvv):
    # [B, HPC, S, DK] -> [B, HPC, S//P, P, 2*DK]: [.., 0:DK]=K^T tile, [.., DK:]=V tile
    b, hpc, s, dk = pk.shape
    kt = pk.reshape(b, hpc, s // P, P, dk).transpose(0, 1, 2, 4, 3)
    vt = pvv.reshape(b, hpc, s // P, P, dk)
    return np.ascontiguousarray(np.concatenate([kt, vt], axis=4))


def _prep_inputs(x, past_key, past_value, Wq, Wk, Wv, Wo):
    x = np.asarray(x, np.float32)
    past_key = np.asarray(past_key, np.float32)
    past_value = np.asarray(past_value, np.float32)
    Wq = np.asarray(Wq, np.float32)
    Wk = np.asarray(Wk, np.float32)
    Wv = np.asarray(Wv, np.float32)
    Wo = np.asarray(Wo, np.float32)

    xT = np.ascontiguousarray(x.transpose(0, 2, 1)).astype(np.float16)
    i = np.arange(P)[:, None]
    j = np.arange(P)[None, :]
    tri = (j >= i).astype(np.float16)  # valid = key_in_tile <= query_in_tile
    ones = np.ones((P, P), np.float16)

    in_maps = []
    for c in range(NCORES):
        e0 = c * EC
        hs = slice(c * HPC, (c + 1) * HPC)
        in_maps.append({
            "xT": xT,
            "wqT": np.ascontiguousarray(Wq[e0:e0 + EC, :].T).astype(np.float16),
            "wkT": np.ascontiguousarray(Wk[e0:e0 + EC, :].T).astype(np.float16),
            "wvT": np.ascontiguousarray(Wv[e0:e0 + EC, :].T).astype(np.float16),
            "woT": np.ascontiguousarray(Wo[:, e0:e0 + EC].T).astype(np.float16),
            "pkv": pack_kv(past_key[:, hs], past_value[:, hs]).astype(np.float16),
            "tri": tri,
            "ones": ones,
        })
    return in_maps


def _run(inputs, trace=False):
    nc = _get_nc()
    in_maps = _prep_inputs(**inputs)
    res = run_bass_kernel_spmd(nc, in_maps, core_ids=list(range(NCORES)), trace=trace)
    total = res.results[0]["out"].astype(np.float32)
    for c in range(1, NCORES):
        total += res.results[c]["out"]
    return total, res


def kernel(x, past_key, past_value, Wq, Wk, Wv, Wo):
    total, _ = _run(dict(x=x, past_key=past_key, past_value=past_value,
                         Wq=Wq, Wk=Wk, Wv=Wv, Wo=Wo))
    return total
